# revision 3
# baseline (speedup 1.0000x reference)
"""Trainium2 kernel for nn_CenterDisc (segment_reduce).

Computes: per-class (4 classes) mean of x rows (N=4096 rows of 64x512),
then mean pairwise Frobenius distance between the 4 class centers.

Strategy (data-parallel over N, 8 cores):
  - host: build one-hot(labels) per shard (tiny), shard x rows 512/core
  - device: per-class partial sums via TensorE matmul
        sums[c, d] = sum_k onehot[k, c] * x[k, d]
    streaming 64 MB/core of x from HBM (memory-bound), one-hot is the
    stationary operand (4 cols -> LDWEIGHTS ~free), accumulation over the
    4 row-chunks of 128 in PSUM.
  - host: add the 8 partial (4, 32768) sums, counts = bincount(labels),
    centers + pairwise norms (tiny) on host.

v2: 2 MiB x DMAs (measured ~420 GB/s vs ~414 at 1 MiB), 4096-wide
column blocks using all 8 PSUM banks, and a narrowed final chunk (last
k-tile split 3584+512) so the tail after the last HBM byte is ~3 us.
"""

import numpy as np

import concourse.bass as bass
import concourse.tile as tile
from concourse import bacc, mybir
from concourse.bass import ts
from concourse.bass_utils import run_bass_kernel_spmd

# Problem shape (hardcoded per contract)
N, C, PDIM = 4096, 64, 512
D = C * PDIM           # 32768 features per row
NCLS = 4               # num classes
CORES = 8
R = N // CORES         # 512 rows per core
KP = 128               # rows per matmul chunk (partition dim)
KC = R // KP           # 4 k-chunks per core
JB = 4096              # feature columns per block (2 MiB DMA per k-chunk)
NB = D // JB           # 8 blocks
MM = 512               # matmul moving free dim (fp32 max / PSUM bank)
JS = JB // MM          # 8 matmul slices per block = all 8 PSUM banks
TAIL1 = JB - MM        # 3584: last block's k=3 tile, part 1
LASTC = (NB - 1) * JB  # col offset of last block

_NC_CACHE = None


def _build_bass():
    nc = bacc.Bacc()
    # float32r: same 4-byte layout as fp32 (host arrays stay np.float32),
    # but the PE streams it ~2x faster than fp32's 4 cycles/row.
    mm_dt = mybir.dt.float32r
    x_in = nc.dram_tensor("x", [R, D], mm_dt, kind="ExternalInput")
    oh_in = nc.dram_tensor("onehot", [R, NCLS], mm_dt,
                           kind="ExternalInput")
    out = nc.dram_tensor("sums", [NCLS, D], mybir.dt.float32,
                         kind="ExternalOutput")

    x_r = x_in[:, :].rearrange("(k p) d -> k p d", p=KP)      # (KC, 128, D)
    oh_r = oh_in[:, :].rearrange("(k p) c -> k p c", p=KP)    # (KC, 128, NCLS)

    with tile.TileContext(nc) as tc:
        with (
            tc.tile_pool(name="ohp", bufs=1) as ohp,
            tc.tile_pool(name="xp", bufs=7) as xp,
            tc.tile_pool(name="tailp", bufs=1) as tailp,
            tc.tile_pool(name="outp", bufs=2) as outp,
            tc.tile_pool(name="tailo", bufs=1) as tailo,
            tc.tile_pool(name="pp", bufs=8, space="PSUM") as pp,
        ):
            ohts = []
            for k in range(KC):
                t = ohp.tile([KP, NCLS], mm_dt, tag=f"oh{k}")
                nc.scalar.dma_start(out=t[:], in_=oh_r[k])
                ohts.append(t)

            for jb in range(NB):
                c0 = jb * JB
                last = jb == NB - 1
                xts = []
                for k in range(KC):
                    if last and k == KC - 1:
                        t1 = tailp.tile([KP, TAIL1], mm_dt, tag="xt1")
                        nc.sync.dma_start(out=t1[:],
                                          in_=x_r[k, :, c0:c0 + TAIL1])
                        t2 = tailp.tile([KP, MM], mm_dt, tag="xt2")
                        nc.sync.dma_start(out=t2[:],
                                          in_=x_r[k, :, c0 + TAIL1:c0 + JB])
                        xts.append((t1, t2))
                    else:
                        xt = xp.tile([KP, JB], mm_dt, tag="x")
                        nc.sync.dma_start(out=xt[:],
                                          in_=x_r[k, :, c0:c0 + JB])
                        xts.append(xt)

                pss = [pp.tile([NCLS, MM], mybir.dt.float32, tag="ps",
                               name=f"ps{jb}_{j}")
                       for j in range(JS)]
                if not last:
                    for k in range(KC):
                        for j in range(JS):
                            nc.tensor.matmul(
                                pss[j][:], ohts[k][:], xts[k][:, ts(j, MM)],
                                start=(k == 0), stop=(k == KC - 1))
                    ot = outp.tile([NCLS, JB], mybir.dt.float32, tag="ot")
                    for j in range(JS):
                        nc.vector.tensor_copy(out=ot[:, ts(j, MM)],
                                              in_=pss[j][:])
                    nc.scalar.dma_start(out=out[:, c0:c0 + JB], in_=ot[:])
                else:
                    # k=0..2 full-width; k=3 split 3584+512 so the final
                    # matmul+copy+store chain hangs off a 256 KiB DMA.
                    for k in range(KC - 1):
                        for j in range(JS):
                            nc.tensor.matmul(
                                pss[j][:], ohts[k][:], xts[k][:, ts(j, MM)],
                                start=(k == 0), stop=False)
                    t1, t2 = xts[KC - 1]
                    for j in range(JS - 1):
                        nc.tensor.matmul(
                            pss[j][:], ohts[KC - 1][:], t1[:, ts(j, MM)],
                            start=False, stop=True)
                    ot1 = tailo.tile([NCLS, TAIL1], mybir.dt.float32,
                                     tag="ot1")
                    for j in range(JS - 1):
                        nc.vector.tensor_copy(out=ot1[:, ts(j, MM)],
                                              in_=pss[j][:])
                    nc.scalar.dma_start(out=out[:, c0:c0 + TAIL1],
                                        in_=ot1[:])
                    nc.tensor.matmul(
                        pss[JS - 1][:], ohts[KC - 1][:], t2[:],
                        start=False, stop=True)
                    ot2 = tailo.tile([NCLS, MM], mybir.dt.float32,
                                     tag="ot2")
                    nc.vector.tensor_copy(out=ot2[:], in_=pss[JS - 1][:])
                    nc.scalar.dma_start(out=out[:, c0 + TAIL1:c0 + JB],
                                        in_=ot2[:])
    nc.compile()
    return nc


def _get_nc():
    global _NC_CACHE
    if _NC_CACHE is None:
        _NC_CACHE = _build_bass()
    return _NC_CACHE


def _run(x, labels, trace=False, **spmd_kwargs):
    x = np.ascontiguousarray(np.asarray(x, dtype=np.float32).reshape(N, D))
    labels = np.asarray(labels).astype(np.int64)
    onehot = (labels[:, None] == np.arange(NCLS)[None, :]).astype(np.float32)

    in_maps = [
        {"x": x[c * R:(c + 1) * R], "onehot": onehot[c * R:(c + 1) * R]}
        for c in range(CORES)
    ]
    nc = _get_nc()
    last_err = None
    for attempt in range(3):
        try:
            br = run_bass_kernel_spmd(nc, in_maps, core_ids=list(range(CORES)),
                                      trace=trace, **spmd_kwargs)
            break
        except Exception as e:  # transient device wedge (NRT_*) — retry
            last_err = e
            import time as _time
            _time.sleep(3.0)
    else:
        raise last_err

    sums = np.zeros((NCLS, D), dtype=np.float64)
    for r in br.results:
        sums += r["sums"].astype(np.float64)
    counts = np.bincount(labels, minlength=NCLS).astype(np.float64)
    safe = np.maximum(counts, 1.0)
    centers = sums / safe[:, None]                         # (NCLS, D)
    diffs = centers[:, None, :] - centers[None, :, :]      # (NCLS, NCLS, D)
    norms = np.sqrt(np.sum(diffs * diffs, axis=-1))        # (NCLS, NCLS)
    iu, ju = np.triu_indices(NCLS, k=1)
    distance = np.sum(norms[iu, ju]) / len(iu)
    return np.asarray(distance, dtype=np.float32), br


def kernel(x, labels):
    result, _ = _run(x, labels, trace=False)
    return result


# revision 4
# speedup vs baseline: 1.1278x; 1.1278x over previous
"""v3: 2 MiB x DMAs for 7x4096-col superblocks; final 4096 cols as two
2048-col mini-blocks, with the very last k-tile split 1536+512 so the
post-stream tail is ~4.5 us (stop-matmuls + dual-engine copies + a tiny
final write on the idle Sync queue)."""

import numpy as np

import concourse.bass as bass
import concourse.tile as tile
from concourse import bacc, mybir
from concourse.bass import ts
from concourse.bass_utils import run_bass_kernel_spmd

N, C, PDIM = 4096, 64, 512
D = C * PDIM           # 32768
NCLS = 4
CORES = 8
R = N // CORES         # 512
KP = 128
KC = R // KP           # 4
SB = 4096              # superblock cols (2 MiB DMA per k-chunk)
NSB = 7                # full superblocks
MM = 512
MB = 2048              # mini-block cols (final 2 blocks)
A0 = NSB * SB          # 28672
B0 = A0 + MB           # 30720
T1 = MB - MM           # 1536

_NC_CACHE = None


def _build_bass():
    nc = bacc.Bacc()
    mm_dt = mybir.dt.float32r
    x_in = nc.dram_tensor("x", [R, D], mm_dt, kind="ExternalInput")
    oh_in = nc.dram_tensor("onehot", [R, NCLS], mm_dt, kind="ExternalInput")
    out = nc.dram_tensor("sums", [NCLS, D], mybir.dt.float32,
                         kind="ExternalOutput")

    x_r = x_in[:, :].rearrange("(k p) d -> k p d", p=KP)
    oh_r = oh_in[:, :].rearrange("(k p) c -> k p c", p=KP)

    with tile.TileContext(nc) as tc:
        with (
            tc.tile_pool(name="ohp", bufs=1) as ohp,
            tc.tile_pool(name="xp", bufs=6) as xp,
            tc.tile_pool(name="xm", bufs=7) as xm,
            tc.tile_pool(name="tailp", bufs=1) as tailp,
            tc.tile_pool(name="outp", bufs=2) as outp,
            tc.tile_pool(name="tailo", bufs=1) as tailo,
            tc.tile_pool(name="pp", bufs=8, space="PSUM") as pp,
        ):
            ohts = []
            for k in range(KC):
                t = ohp.tile([KP, NCLS], mm_dt, tag=f"oh{k}")
                nc.scalar.dma_start(out=t[:], in_=oh_r[k])
                ohts.append(t)

            def evict(j, dst, src):
                # alternate DVE / ACT so eviction chains halve
                if j % 2 == 0:
                    nc.vector.tensor_copy(out=dst, in_=src)
                else:
                    nc.scalar.copy(out=dst, in_=src)

            # --- 7 full superblocks of 4096 cols ---
            for sb in range(NSB):
                c0 = sb * SB
                xts = []
                for k in range(KC):
                    xt = xp.tile([KP, SB], mm_dt, tag="x", name=f"x{sb}_{k}")
                    nc.sync.dma_start(out=xt[:], in_=x_r[k, :, c0:c0 + SB])
                    xts.append(xt)
                pss = [pp.tile([NCLS, MM], mybir.dt.float32, tag="ps",
                               name=f"ps{sb}_{j}") for j in range(8)]
                for k in range(KC):
                    for j in range(8):
                        nc.tensor.matmul(
                            pss[j][:], ohts[k][:], xts[k][:, ts(j, MM)],
                            start=(k == 0), stop=(k == KC - 1))
                ot = outp.tile([NCLS, SB], mybir.dt.float32, tag="ot",
                               name=f"ot{sb}")
                for j in range(8):
                    evict(j, ot[:, ts(j, MM)], pss[j][:])
                nc.scalar.dma_start(out=out[:, c0:c0 + SB], in_=ot[:])

            # --- mini-block A: cols 28672..30719 ---
            axts = []
            for k in range(KC):
                xt = xm.tile([KP, MB], mm_dt, tag="xm", name=f"xa{k}")
                nc.sync.dma_start(out=xt[:], in_=x_r[k, :, A0:A0 + MB])
                axts.append(xt)
            # --- mini-block B DMAs: k0..k2 full, k3 split 1536+512 ---
            bxts = []
            for k in range(KC - 1):
                xt = xm.tile([KP, MB], mm_dt, tag="xm", name=f"xb{k}")
                nc.sync.dma_start(out=xt[:], in_=x_r[k, :, B0:B0 + MB])
                bxts.append(xt)
            bt1 = tailp.tile([KP, T1], mm_dt, tag="xt1")
            nc.sync.dma_start(out=bt1[:], in_=x_r[KC - 1, :, B0:B0 + T1])
            bt2 = tailp.tile([KP, MM], mm_dt, tag="xt2")
            nc.sync.dma_start(out=bt2[:], in_=x_r[KC - 1, :, B0 + T1:B0 + MB])

            # A compute: 4 banks
            psa = [pp.tile([NCLS, MM], mybir.dt.float32, tag="ps",
                           name=f"psa{j}") for j in range(4)]
            for k in range(KC):
                for j in range(4):
                    nc.tensor.matmul(
                        psa[j][:], ohts[k][:], axts[k][:, ts(j, MM)],
                        start=(k == 0), stop=(k == KC - 1))
            ota = outp.tile([NCLS, SB], mybir.dt.float32, tag="ot",
                            name="ota")
            for j in range(4):
                evict(j, ota[:, ts(j, MM)], psa[j][:])
            nc.scalar.dma_start(out=out[:, A0:A0 + MB], in_=ota[:, 0:MB])

            # B compute: 4 banks; k3 stops split across bt1 (j0..2) / bt2 (j3)
            psb = [pp.tile([NCLS, MM], mybir.dt.float32, tag="ps",
                           name=f"psb{j}") for j in range(4)]
            for k in range(KC - 1):
                for j in range(4):
                    nc.tensor.matmul(
                        psb[j][:], ohts[k][:], bxts[k][:, ts(j, MM)],
                        start=(k == 0), stop=False)
            for j in range(3):
                nc.tensor.matmul(
                    psb[j][:], ohts[KC - 1][:], bt1[:, ts(j, MM)],
                    start=False, stop=True)
            nc.tensor.matmul(
                psb[3][:], ohts[KC - 1][:], bt2[:],
                start=False, stop=True)
            otb = tailo.tile([NCLS, T1], mybir.dt.float32, tag="otb")
            nc.vector.tensor_copy(out=otb[:, 0:MM], in_=psb[0][:])
            nc.scalar.copy(out=otb[:, ts(1, MM)], in_=psb[1][:])
            nc.vector.tensor_copy(out=otb[:, ts(2, MM)], in_=psb[2][:])
            nc.scalar.dma_start(out=out[:, B0:B0 + T1], in_=otb[:])
            otb2 = tailo.tile([NCLS, MM], mybir.dt.float32, tag="otb2")
            nc.vector.tensor_copy(out=otb2[:], in_=psb[3][:])
            nc.sync.dma_start(out=out[:, B0 + T1:B0 + MB], in_=otb2[:])
    nc.compile()
    return nc


def _get_nc():
    global _NC_CACHE
    if _NC_CACHE is None:
        _NC_CACHE = _build_bass()
    return _NC_CACHE


def _run(x, labels, trace=False, **spmd_kwargs):
    x = np.ascontiguousarray(np.asarray(x, dtype=np.float32).reshape(N, D))
    labels = np.asarray(labels).astype(np.int64)
    onehot = (labels[:, None] == np.arange(NCLS)[None, :]).astype(np.float32)

    in_maps = [
        {"x": x[c * R:(c + 1) * R], "onehot": onehot[c * R:(c + 1) * R]}
        for c in range(CORES)
    ]
    nc = _get_nc()
    last_err = None
    for attempt in range(3):
        try:
            br = run_bass_kernel_spmd(nc, in_maps, core_ids=list(range(CORES)),
                                      trace=trace, **spmd_kwargs)
            break
        except Exception as e:
            last_err = e
            import time as _time
            _time.sleep(3.0)
    else:
        raise last_err

    sums = np.zeros((NCLS, D), dtype=np.float64)
    for r in br.results:
        sums += r["sums"].astype(np.float64)
    counts = np.bincount(labels, minlength=NCLS).astype(np.float64)
    safe = np.maximum(counts, 1.0)
    centers = sums / safe[:, None]
    diffs = centers[:, None, :] - centers[None, :, :]
    norms = np.sqrt(np.sum(diffs * diffs, axis=-1))
    iu, ju = np.triu_indices(NCLS, k=1)
    distance = np.sum(norms[iu, ju]) / len(iu)
    return np.asarray(distance, dtype=np.float32), br


def kernel(x, labels):
    result, _ = _run(x, labels, trace=False)
    return result


# revision 5
# speedup vs baseline: 1.1339x; 1.0054x over previous
"""v4: v3 + tail polish — B's copies alternate DVE/ACT strictly
(j3 on ACT so it never queues behind DVE), and both final writes issue
from the Sync queue, which is idle once the x stream ends."""

import numpy as np

import concourse.bass as bass
import concourse.tile as tile
from concourse import bacc, mybir
from concourse.bass import ts
from concourse.bass_utils import run_bass_kernel_spmd

N, C, PDIM = 4096, 64, 512
D = C * PDIM           # 32768
NCLS = 4
CORES = 8
R = N // CORES         # 512
KP = 128
KC = R // KP           # 4
SB = 4096              # superblock cols (2 MiB DMA per k-chunk)
NSB = 7                # full superblocks
MM = 512
MB = 2048              # mini-block cols (final 2 blocks)
A0 = NSB * SB          # 28672
B0 = A0 + MB           # 30720
T1 = MB - MM           # 1536

_NC_CACHE = None


def _build_bass():
    nc = bacc.Bacc()
    mm_dt = mybir.dt.float32r
    x_in = nc.dram_tensor("x", [R, D], mm_dt, kind="ExternalInput")
    oh_in = nc.dram_tensor("onehot", [R, NCLS], mm_dt, kind="ExternalInput")
    out = nc.dram_tensor("sums", [NCLS, D], mybir.dt.float32,
                         kind="ExternalOutput")

    x_r = x_in[:, :].rearrange("(k p) d -> k p d", p=KP)
    oh_r = oh_in[:, :].rearrange("(k p) c -> k p c", p=KP)

    with tile.TileContext(nc) as tc:
        with (
            tc.tile_pool(name="ohp", bufs=1) as ohp,
            tc.tile_pool(name="xp", bufs=6) as xp,
            tc.tile_pool(name="xm", bufs=7) as xm,
            tc.tile_pool(name="tailp", bufs=1) as tailp,
            tc.tile_pool(name="outp", bufs=2) as outp,
            tc.tile_pool(name="tailo", bufs=1) as tailo,
            tc.tile_pool(name="pp", bufs=8, space="PSUM") as pp,
        ):
            ohts = []
            for k in range(KC):
                t = ohp.tile([KP, NCLS], mm_dt, tag=f"oh{k}")
                nc.scalar.dma_start(out=t[:], in_=oh_r[k])
                ohts.append(t)

            def evict(j, dst, src):
                # alternate DVE / ACT so eviction chains halve
                if j % 2 == 0:
                    nc.vector.tensor_copy(out=dst, in_=src)
                else:
                    nc.scalar.copy(out=dst, in_=src)

            # --- 7 full superblocks of 4096 cols ---
            for sb in range(NSB):
                c0 = sb * SB
                xts = []
                for k in range(KC):
                    xt = xp.tile([KP, SB], mm_dt, tag="x", name=f"x{sb}_{k}")
                    nc.sync.dma_start(out=xt[:], in_=x_r[k, :, c0:c0 + SB])
                    xts.append(xt)
                pss = [pp.tile([NCLS, MM], mybir.dt.float32, tag="ps",
                               name=f"ps{sb}_{j}") for j in range(8)]
                for k in range(KC):
                    for j in range(8):
                        nc.tensor.matmul(
                            pss[j][:], ohts[k][:], xts[k][:, ts(j, MM)],
                            start=(k == 0), stop=(k == KC - 1))
                ot = outp.tile([NCLS, SB], mybir.dt.float32, tag="ot",
                               name=f"ot{sb}")
                for j in range(8):
                    evict(j, ot[:, ts(j, MM)], pss[j][:])
                nc.scalar.dma_start(out=out[:, c0:c0 + SB], in_=ot[:])

            # --- mini-block A: cols 28672..30719 ---
            axts = []
            for k in range(KC):
                xt = xm.tile([KP, MB], mm_dt, tag="xm", name=f"xa{k}")
                nc.sync.dma_start(out=xt[:], in_=x_r[k, :, A0:A0 + MB])
                axts.append(xt)
            # --- mini-block B DMAs: k0..k2 full, k3 split 1536+512 ---
            bxts = []
            for k in range(KC - 1):
                xt = xm.tile([KP, MB], mm_dt, tag="xm", name=f"xb{k}")
                nc.sync.dma_start(out=xt[:], in_=x_r[k, :, B0:B0 + MB])
                bxts.append(xt)
            bt1 = tailp.tile([KP, T1], mm_dt, tag="xt1")
            nc.sync.dma_start(out=bt1[:], in_=x_r[KC - 1, :, B0:B0 + T1])
            bt2 = tailp.tile([KP, MM], mm_dt, tag="xt2")
            nc.sync.dma_start(out=bt2[:], in_=x_r[KC - 1, :, B0 + T1:B0 + MB])

            # A compute: 4 banks
            psa = [pp.tile([NCLS, MM], mybir.dt.float32, tag="ps",
                           name=f"psa{j}") for j in range(4)]
            for k in range(KC):
                for j in range(4):
                    nc.tensor.matmul(
                        psa[j][:], ohts[k][:], axts[k][:, ts(j, MM)],
                        start=(k == 0), stop=(k == KC - 1))
            ota = outp.tile([NCLS, SB], mybir.dt.float32, tag="ot",
                            name="ota")
            for j in range(4):
                evict(j, ota[:, ts(j, MM)], psa[j][:])
            nc.scalar.dma_start(out=out[:, A0:A0 + MB], in_=ota[:, 0:MB])

            # B compute: 4 banks; k3 stops split across bt1 (j0..2) / bt2 (j3)
            psb = [pp.tile([NCLS, MM], mybir.dt.float32, tag="ps",
                           name=f"psb{j}") for j in range(4)]
            for k in range(KC - 1):
                for j in range(4):
                    nc.tensor.matmul(
                        psb[j][:], ohts[k][:], bxts[k][:, ts(j, MM)],
                        start=(k == 0), stop=False)
            for j in range(3):
                nc.tensor.matmul(
                    psb[j][:], ohts[KC - 1][:], bt1[:, ts(j, MM)],
                    start=False, stop=True)
            nc.tensor.matmul(
                psb[3][:], ohts[KC - 1][:], bt2[:],
                start=False, stop=True)
            otb = tailo.tile([NCLS, T1], mybir.dt.float32, tag="otb")
            otb2 = tailo.tile([NCLS, MM], mybir.dt.float32, tag="otb2")
            nc.vector.tensor_copy(out=otb[:, 0:MM], in_=psb[0][:])
            nc.scalar.copy(out=otb[:, ts(1, MM)], in_=psb[1][:])
            nc.vector.tensor_copy(out=otb[:, ts(2, MM)], in_=psb[2][:])
            nc.scalar.copy(out=otb2[:], in_=psb[3][:])
            nc.sync.dma_start(out=out[:, B0:B0 + T1], in_=otb[:])
            nc.sync.dma_start(out=out[:, B0 + T1:B0 + MB], in_=otb2[:])
    nc.compile()
    return nc


def _get_nc():
    global _NC_CACHE
    if _NC_CACHE is None:
        _NC_CACHE = _build_bass()
    return _NC_CACHE


def _run(x, labels, trace=False, **spmd_kwargs):
    x = np.ascontiguousarray(np.asarray(x, dtype=np.float32).reshape(N, D))
    labels = np.asarray(labels).astype(np.int64)
    onehot = (labels[:, None] == np.arange(NCLS)[None, :]).astype(np.float32)

    in_maps = [
        {"x": x[c * R:(c + 1) * R], "onehot": onehot[c * R:(c + 1) * R]}
        for c in range(CORES)
    ]
    nc = _get_nc()
    last_err = None
    for attempt in range(3):
        try:
            br = run_bass_kernel_spmd(nc, in_maps, core_ids=list(range(CORES)),
                                      trace=trace, **spmd_kwargs)
            break
        except Exception as e:
            last_err = e
            import time as _time
            _time.sleep(3.0)
    else:
        raise last_err

    sums = np.zeros((NCLS, D), dtype=np.float64)
    for r in br.results:
        sums += r["sums"].astype(np.float64)
    counts = np.bincount(labels, minlength=NCLS).astype(np.float64)
    safe = np.maximum(counts, 1.0)
    centers = sums / safe[:, None]
    diffs = centers[:, None, :] - centers[None, :, :]
    norms = np.sqrt(np.sum(diffs * diffs, axis=-1))
    iu, ju = np.triu_indices(NCLS, k=1)
    distance = np.sum(norms[iu, ju]) / len(iu)
    return np.asarray(distance, dtype=np.float32), br


def kernel(x, labels):
    result, _ = _run(x, labels, trace=False)
    return result


# revision 6
# speedup vs baseline: 1.1460x; 1.0107x over previous
"""v7: v4 + bf16 output path — PSUM (fp32) evictions cast to bf16 on
the copy, halving the 512 KB of mid-stream out-writes that steal SDMA
time from the x stream, and halving tail write payloads. Host upcasts;
bf16 partial sums cost ~1.4e-6 rel err."""

import numpy as np

import concourse.bass as bass
import concourse.tile as tile
from concourse import bacc, mybir
from concourse.bass import ts
from concourse.bass_utils import run_bass_kernel_spmd

N, C, PDIM = 4096, 64, 512
D = C * PDIM           # 32768
NCLS = 4
CORES = 8
R = N // CORES         # 512
KP = 128
KC = R // KP           # 4
SB = 4096              # superblock cols (2 MiB DMA per k-chunk)
NSB = 7                # full superblocks
MM = 512
MB = 2048              # mini-block cols (final 2 blocks)
A0 = NSB * SB          # 28672
B0 = A0 + MB           # 30720
T1 = MB - MM           # 1536

_NC_CACHE = None


def _build_bass():
    nc = bacc.Bacc()
    mm_dt = mybir.dt.float32r
    x_in = nc.dram_tensor("x", [R, D], mm_dt, kind="ExternalInput")
    oh_in = nc.dram_tensor("onehot", [R, NCLS], mm_dt, kind="ExternalInput")
    out = nc.dram_tensor("sums", [NCLS, D], mybir.dt.bfloat16,
                         kind="ExternalOutput")

    x_r = x_in[:, :].rearrange("(k p) d -> k p d", p=KP)
    oh_r = oh_in[:, :].rearrange("(k p) c -> k p c", p=KP)

    with tile.TileContext(nc) as tc:
        with (
            tc.tile_pool(name="ohp", bufs=1) as ohp,
            tc.tile_pool(name="xp", bufs=6) as xp,
            tc.tile_pool(name="xm", bufs=7) as xm,
            tc.tile_pool(name="tailp", bufs=1) as tailp,
            tc.tile_pool(name="outp", bufs=2) as outp,
            tc.tile_pool(name="tailo", bufs=1) as tailo,
            tc.tile_pool(name="pp", bufs=8, space="PSUM") as pp,
        ):
            ohts = []
            for k in range(KC):
                t = ohp.tile([KP, NCLS], mm_dt, tag=f"oh{k}")
                nc.scalar.dma_start(out=t[:], in_=oh_r[k])
                ohts.append(t)

            def evict(j, dst, src):
                # alternate DVE / ACT so eviction chains halve
                if j % 2 == 0:
                    nc.vector.tensor_copy(out=dst, in_=src)
                else:
                    nc.scalar.copy(out=dst, in_=src)

            # --- 7 full superblocks of 4096 cols ---
            for sb in range(NSB):
                c0 = sb * SB
                xts = []
                for k in range(KC):
                    xt = xp.tile([KP, SB], mm_dt, tag="x", name=f"x{sb}_{k}")
                    nc.sync.dma_start(out=xt[:], in_=x_r[k, :, c0:c0 + SB])
                    xts.append(xt)
                pss = [pp.tile([NCLS, MM], mybir.dt.float32, tag="ps",
                               name=f"ps{sb}_{j}") for j in range(8)]
                for k in range(KC):
                    for j in range(8):
                        nc.tensor.matmul(
                            pss[j][:], ohts[k][:], xts[k][:, ts(j, MM)],
                            start=(k == 0), stop=(k == KC - 1))
                ot = outp.tile([NCLS, SB], mybir.dt.bfloat16, tag="ot",
                               name=f"ot{sb}")
                for j in range(8):
                    evict(j, ot[:, ts(j, MM)], pss[j][:])
                nc.scalar.dma_start(out=out[:, c0:c0 + SB], in_=ot[:])

            # --- mini-block A: cols 28672..30719 ---
            axts = []
            for k in range(KC):
                xt = xm.tile([KP, MB], mm_dt, tag="xm", name=f"xa{k}")
                nc.sync.dma_start(out=xt[:], in_=x_r[k, :, A0:A0 + MB])
                axts.append(xt)
            # --- mini-block B DMAs: k0..k2 full, k3 split 1536+512 ---
            bxts = []
            for k in range(KC - 1):
                xt = xm.tile([KP, MB], mm_dt, tag="xm", name=f"xb{k}")
                nc.sync.dma_start(out=xt[:], in_=x_r[k, :, B0:B0 + MB])
                bxts.append(xt)
            bt1 = tailp.tile([KP, T1], mm_dt, tag="xt1")
            nc.sync.dma_start(out=bt1[:], in_=x_r[KC - 1, :, B0:B0 + T1])
            bt2 = tailp.tile([KP, MM], mm_dt, tag="xt2")
            nc.sync.dma_start(out=bt2[:], in_=x_r[KC - 1, :, B0 + T1:B0 + MB])

            # A compute: 4 banks
            psa = [pp.tile([NCLS, MM], mybir.dt.float32, tag="ps",
                           name=f"psa{j}") for j in range(4)]
            for k in range(KC):
                for j in range(4):
                    nc.tensor.matmul(
                        psa[j][:], ohts[k][:], axts[k][:, ts(j, MM)],
                        start=(k == 0), stop=(k == KC - 1))
            ota = outp.tile([NCLS, SB], mybir.dt.bfloat16, tag="ot",
                            name="ota")
            for j in range(4):
                evict(j, ota[:, ts(j, MM)], psa[j][:])
            nc.scalar.dma_start(out=out[:, A0:A0 + MB], in_=ota[:, 0:MB])

            # B compute: 4 banks; k3 stops split across bt1 (j0..2) / bt2 (j3)
            psb = [pp.tile([NCLS, MM], mybir.dt.float32, tag="ps",
                           name=f"psb{j}") for j in range(4)]
            for k in range(KC - 1):
                for j in range(4):
                    nc.tensor.matmul(
                        psb[j][:], ohts[k][:], bxts[k][:, ts(j, MM)],
                        start=(k == 0), stop=False)
            for j in range(3):
                nc.tensor.matmul(
                    psb[j][:], ohts[KC - 1][:], bt1[:, ts(j, MM)],
                    start=False, stop=True)
            nc.tensor.matmul(
                psb[3][:], ohts[KC - 1][:], bt2[:],
                start=False, stop=True)
            otb = tailo.tile([NCLS, T1], mybir.dt.bfloat16, tag="otb")
            otb2 = tailo.tile([NCLS, MM], mybir.dt.bfloat16, tag="otb2")
            nc.vector.tensor_copy(out=otb[:, 0:MM], in_=psb[0][:])
            nc.scalar.copy(out=otb[:, ts(1, MM)], in_=psb[1][:])
            nc.vector.tensor_copy(out=otb[:, ts(2, MM)], in_=psb[2][:])
            nc.scalar.copy(out=otb2[:], in_=psb[3][:])
            nc.sync.dma_start(out=out[:, B0:B0 + T1], in_=otb[:])
            nc.sync.dma_start(out=out[:, B0 + T1:B0 + MB], in_=otb2[:])
    nc.compile()
    return nc


def _get_nc():
    global _NC_CACHE
    if _NC_CACHE is None:
        _NC_CACHE = _build_bass()
    return _NC_CACHE


def _run(x, labels, trace=False, **spmd_kwargs):
    x = np.ascontiguousarray(np.asarray(x, dtype=np.float32).reshape(N, D))
    labels = np.asarray(labels).astype(np.int64)
    onehot = (labels[:, None] == np.arange(NCLS)[None, :]).astype(np.float32)

    in_maps = [
        {"x": x[c * R:(c + 1) * R], "onehot": onehot[c * R:(c + 1) * R]}
        for c in range(CORES)
    ]
    nc = _get_nc()
    last_err = None
    for attempt in range(3):
        try:
            br = run_bass_kernel_spmd(nc, in_maps, core_ids=list(range(CORES)),
                                      trace=trace, **spmd_kwargs)
            break
        except Exception as e:
            last_err = e
            import time as _time
            _time.sleep(3.0)
    else:
        raise last_err

    sums = np.zeros((NCLS, D), dtype=np.float64)
    for r in br.results:
        sums += r["sums"].astype(np.float64)
    counts = np.bincount(labels, minlength=NCLS).astype(np.float64)
    safe = np.maximum(counts, 1.0)
    centers = sums / safe[:, None]
    diffs = centers[:, None, :] - centers[None, :, :]
    norms = np.sqrt(np.sum(diffs * diffs, axis=-1))
    iu, ju = np.triu_indices(NCLS, k=1)
    distance = np.sum(norms[iu, ju]) / len(iu)
    return np.asarray(distance, dtype=np.float32), br


def kernel(x, labels):
    result, _ = _run(x, labels, trace=False)
    return result


# revision 7
# speedup vs baseline: 1.1501x; 1.0036x over previous
"""v7: v4 + bf16 output path — PSUM (fp32) evictions cast to bf16 on
the copy, halving the 512 KB of mid-stream out-writes that steal SDMA
time from the x stream, and halving tail write payloads. Host upcasts;
bf16 partial sums cost ~1.4e-6 rel err."""

import numpy as np

import concourse.bass as bass
import concourse.tile as tile
from concourse import bacc, mybir
from concourse.bass import ts
from concourse.bass_utils import run_bass_kernel_spmd

N, C, PDIM = 4096, 64, 512
D = C * PDIM           # 32768
NCLS = 4
CORES = 8
R = N // CORES         # 512
KP = 128
KC = R // KP           # 4
SB = 4096              # superblock cols (2 MiB DMA per k-chunk)
NSB = 7                # full superblocks
MM = 512
MB = 2048              # mini-block cols (final 2 blocks)
A0 = NSB * SB          # 28672
B0 = A0 + MB           # 30720
T1 = MB - MM           # 1536

_NC_CACHE = None


def _build_bass():
    nc = bacc.Bacc()
    mm_dt = mybir.dt.float32r
    x_in = nc.dram_tensor("x", [R, D], mm_dt, kind="ExternalInput")
    oh_in = nc.dram_tensor("onehot", [R, NCLS], mm_dt, kind="ExternalInput")
    out = nc.dram_tensor("sums", [NCLS, D], mybir.dt.bfloat16,
                         kind="ExternalOutput")

    x_r = x_in[:, :].rearrange("(k p) d -> k p d", p=KP)
    oh_r = oh_in[:, :].rearrange("(k p) c -> k p c", p=KP)

    with tile.TileContext(nc) as tc:
        with (
            tc.tile_pool(name="ohp", bufs=1) as ohp,
            tc.tile_pool(name="xp", bufs=6) as xp,
            tc.tile_pool(name="xm", bufs=7) as xm,
            tc.tile_pool(name="tailp", bufs=1) as tailp,
            tc.tile_pool(name="outp", bufs=2) as outp,
            tc.tile_pool(name="tailo", bufs=1) as tailo,
            tc.tile_pool(name="pp", bufs=8, space="PSUM") as pp,
        ):
            ohts = []
            for k in range(KC):
                t = ohp.tile([KP, NCLS], mm_dt, tag=f"oh{k}")
                nc.scalar.dma_start(out=t[:], in_=oh_r[k])
                ohts.append(t)

            def evict(j, dst, src):
                # alternate DVE / ACT so eviction chains halve
                if j % 2 == 0:
                    nc.vector.tensor_copy(out=dst, in_=src)
                else:
                    nc.scalar.copy(out=dst, in_=src)

            # --- 7 full superblocks of 4096 cols ---
            for sb in range(NSB):
                c0 = sb * SB
                xts = []
                for k in range(KC):
                    xt = xp.tile([KP, SB], mm_dt, tag="x", name=f"x{sb}_{k}")
                    nc.sync.dma_start(out=xt[:], in_=x_r[k, :, c0:c0 + SB])
                    xts.append(xt)
                pss = [pp.tile([NCLS, MM], mybir.dt.float32, tag="ps",
                               name=f"ps{sb}_{j}") for j in range(8)]
                for k in range(KC):
                    for j in range(8):
                        nc.tensor.matmul(
                            pss[j][:], ohts[k][:], xts[k][:, ts(j, MM)],
                            start=(k == 0), stop=(k == KC - 1))
                ot = outp.tile([NCLS, SB], mybir.dt.bfloat16, tag="ot",
                               name=f"ot{sb}")
                for j in range(8):
                    evict(j, ot[:, ts(j, MM)], pss[j][:])
                nc.scalar.dma_start(out=out[:, c0:c0 + SB], in_=ot[:])

            # --- mini-block A: cols 28672..30719 ---
            axts = []
            for k in range(KC):
                xt = xm.tile([KP, MB], mm_dt, tag="xm", name=f"xa{k}")
                nc.sync.dma_start(out=xt[:], in_=x_r[k, :, A0:A0 + MB])
                axts.append(xt)
            # --- mini-block B DMAs: k0..k2 full, k3 split 1536+512 ---
            bxts = []
            for k in range(KC - 1):
                xt = xm.tile([KP, MB], mm_dt, tag="xm", name=f"xb{k}")
                nc.sync.dma_start(out=xt[:], in_=x_r[k, :, B0:B0 + MB])
                bxts.append(xt)
            bt1 = tailp.tile([KP, T1], mm_dt, tag="xt1")
            nc.sync.dma_start(out=bt1[:], in_=x_r[KC - 1, :, B0:B0 + T1])
            bt2 = tailp.tile([KP, MM], mm_dt, tag="xt2")
            nc.sync.dma_start(out=bt2[:], in_=x_r[KC - 1, :, B0 + T1:B0 + MB])

            # A compute: 4 banks
            psa = [pp.tile([NCLS, MM], mybir.dt.float32, tag="ps",
                           name=f"psa{j}") for j in range(4)]
            for k in range(KC):
                for j in range(4):
                    nc.tensor.matmul(
                        psa[j][:], ohts[k][:], axts[k][:, ts(j, MM)],
                        start=(k == 0), stop=(k == KC - 1))
            ota = outp.tile([NCLS, SB], mybir.dt.bfloat16, tag="ot",
                            name="ota")
            for j in range(4):
                evict(j, ota[:, ts(j, MM)], psa[j][:])
            nc.scalar.dma_start(out=out[:, A0:A0 + MB], in_=ota[:, 0:MB])

            # B compute: 4 banks; k3 stops split across bt1 (j0..2) / bt2 (j3)
            psb = [pp.tile([NCLS, MM], mybir.dt.float32, tag="ps",
                           name=f"psb{j}") for j in range(4)]
            for k in range(KC - 1):
                for j in range(4):
                    nc.tensor.matmul(
                        psb[j][:], ohts[k][:], bxts[k][:, ts(j, MM)],
                        start=(k == 0), stop=False)
            for j in range(3):
                nc.tensor.matmul(
                    psb[j][:], ohts[KC - 1][:], bt1[:, ts(j, MM)],
                    start=False, stop=True)
            nc.tensor.matmul(
                psb[3][:], ohts[KC - 1][:], bt2[:],
                start=False, stop=True)
            otb = tailo.tile([NCLS, T1], mybir.dt.bfloat16, tag="otb")
            otb2 = tailo.tile([NCLS, MM], mybir.dt.bfloat16, tag="otb2")
            nc.vector.tensor_copy(out=otb[:, 0:MM], in_=psb[0][:])
            nc.scalar.copy(out=otb[:, ts(1, MM)], in_=psb[1][:])
            nc.vector.tensor_copy(out=otb[:, ts(2, MM)], in_=psb[2][:])
            nc.scalar.copy(out=otb2[:], in_=psb[3][:])
            # otb (needs j0..2) issues from Scalar in parallel with the
            # final otb2 issue on Sync — serializing both on Sync costs
            # ~0.4 us at the very end of the kernel.
            nc.scalar.dma_start(out=out[:, B0:B0 + T1], in_=otb[:])
            nc.sync.dma_start(out=out[:, B0 + T1:B0 + MB], in_=otb2[:])
    nc.compile()
    return nc


def _get_nc():
    global _NC_CACHE
    if _NC_CACHE is None:
        _NC_CACHE = _build_bass()
    return _NC_CACHE


def _run(x, labels, trace=False, **spmd_kwargs):
    x = np.ascontiguousarray(np.asarray(x, dtype=np.float32).reshape(N, D))
    labels = np.asarray(labels).astype(np.int64)
    onehot = (labels[:, None] == np.arange(NCLS)[None, :]).astype(np.float32)

    in_maps = [
        {"x": x[c * R:(c + 1) * R], "onehot": onehot[c * R:(c + 1) * R]}
        for c in range(CORES)
    ]
    nc = _get_nc()
    last_err = None
    for attempt in range(3):
        try:
            br = run_bass_kernel_spmd(nc, in_maps, core_ids=list(range(CORES)),
                                      trace=trace, **spmd_kwargs)
            break
        except Exception as e:
            last_err = e
            import time as _time
            _time.sleep(3.0)
    else:
        raise last_err

    sums = np.zeros((NCLS, D), dtype=np.float64)
    for r in br.results:
        sums += r["sums"].astype(np.float64)
    counts = np.bincount(labels, minlength=NCLS).astype(np.float64)
    safe = np.maximum(counts, 1.0)
    centers = sums / safe[:, None]
    diffs = centers[:, None, :] - centers[None, :, :]
    norms = np.sqrt(np.sum(diffs * diffs, axis=-1))
    iu, ju = np.triu_indices(NCLS, k=1)
    distance = np.sum(norms[iu, ju]) / len(iu)
    return np.asarray(distance, dtype=np.float32), br


def kernel(x, labels):
    result, _ = _run(x, labels, trace=False)
    return result
